# revision 23
# baseline (speedup 1.0000x reference)
"""Multi-head causal attention (B=2, S=2048, D=1024, H=16) on 8 trn2 cores.

Sharding (Megatron TP over batch*heads): core c handles batch c//4 and the
4 heads 4*(c%4)..4*(c%4)+3.  Wq/Wk/Wv are column-sharded (each core gets the
256 rows of W* for its heads), Wo is row-sharded; the host sums the 4 partial
outputs per batch (the tensor-parallel all-reduce) and adds bo.

The schedule is built around the ScalarE exp stream (the bottleneck, ~110us
of activations): projections are column-halved so the first score tiles exist
as soon as the first halves of q/k stream in, attention runs t-chunk-outer /
head-pair-inner, and psum->sbuf drains run on DVE/gpsimd (except pre-exp and
post-exp windows where ScalarE is idle anyway).  PSUM is four 2-bank slots:
P0/P1 rotate score tiles, P2/P3 hold attn@V accumulators and v/late
projection groups between accumulations.  Softmax normalization keeps the
psum critical path short (raw copies only) and does the reciprocal via a
32-lane transpose dance (DVE reciprocal costs ~8ns/elem per partition);
the final scale runs on gpsimd off every critical engine.
"""

import os
import sys

sys.path.insert(0, "/opt/trn_rl_repo")

import ml_dtypes
import numpy as np

import concourse.bass as bass  # noqa: F401
import concourse.bass2jax as bass2jax
import concourse.tile as tile
from concourse import bacc, mybir
from concourse.bass_utils import run_bass_kernel_spmd

# Optional NEFF tee for local profiling (active only when the env var is set).
_orig_rename = bass2jax.rename_neff_tensors_and_patch_header


def _tee_rename(neff_path, mapping):
    data = _orig_rename(neff_path, mapping)
    tee = os.environ.get("BASS_MHA_NEFF_TEE")
    if tee:
        try:
            with open(tee, "wb") as f:
                f.write(data)
        except OSError:
            pass
    return data


bass2jax.rename_neff_tensors_and_patch_header = _tee_rename

F32 = mybir.dt.float32
AF = mybir.ActivationFunctionType

S = 2048  # sequence length
D = 1024  # model dim
HL = 256  # local head width (4 heads x 64)
DK = 64  # head dim
N_SI = S // 128  # 16 key tiles

MODE = os.environ.get("BASS_MHA_DTYPE", "bf16")  # bf16 | f32r | f32
if MODE == "bf16":
    DT = mybir.dt.bfloat16
    NPDT = ml_dtypes.bfloat16
elif MODE == "f32r":
    DT = mybir.dt.float32r
    NPDT = np.float32
else:
    DT = F32
    NPDT = np.float32

LAST_EXEC_NS = None
_CACHED_NC = None


def _prep(a):
    """Cast a host array to the kernel's compute dtype (with fp32r rounding
    matching the compiler's fp32_to_fp32r when in f32r mode)."""
    a = np.ascontiguousarray(np.asarray(a, np.float32))
    if MODE == "bf16":
        return a.astype(ml_dtypes.bfloat16)
    if MODE == "f32r":
        bits = a.view(np.uint32).astype(np.uint64)
        rounded = (bits + 0x7FF + ((bits >> 12) & 1)) & 0xFFFFF000
        return rounded.astype(np.uint32).view(np.float32).reshape(a.shape)
    return a


def _prep_w(w):
    """[n*128, c] weight -> [128, n, c] partition-major layout (4KB DMA rows)."""
    return _prep(
        np.ascontiguousarray(w.reshape(-1, 128, w.shape[-1]).transpose(1, 0, 2))
    )


def _build_kernel(tc):
    nc = tc.nc
    qt = nc.dram_tensor("qt", [D, S], DT, kind="ExternalInput").ap()
    kt = nc.dram_tensor("kt", [D, S], DT, kind="ExternalInput").ap()
    vt = nc.dram_tensor("vt", [D, S], DT, kind="ExternalInput").ap()
    wqt = nc.dram_tensor("wqt", [128, 8, HL], DT, kind="ExternalInput").ap()
    wkt = nc.dram_tensor("wkt", [128, 8, HL], DT, kind="ExternalInput").ap()
    wvt = nc.dram_tensor("wvt", [128, 8, HL], DT, kind="ExternalInput").ap()
    wot = nc.dram_tensor("wot", [128, 2, D], DT, kind="ExternalInput").ap()
    mtri = nc.dram_tensor("mtri", [128, 128], DT, kind="ExternalInput").ap()
    out = nc.dram_tensor("out", [S, D], DT, kind="ExternalOutput").ap()

    consts = tc.alloc_tile_pool(name="consts", bufs=1)
    persist = tc.alloc_tile_pool(name="persist", bufs=1)
    xt_pool = tc.alloc_tile_pool(name="xt", bufs=16)
    attn_pool = tc.alloc_tile_pool(name="attn", bufs=22)
    bc_pool = tc.alloc_tile_pool(name="bc", bufs=4)
    out_pool = tc.alloc_tile_pool(name="outsb", bufs=4)
    psum = tc.alloc_tile_pool(name="ps", bufs=1, space="PSUM")

    # --- weights in SBUF (already partition-major in dram: 4KB rows) ---
    wq_sb = consts.tile([128, 8, HL], DT, name="wq_sb")
    wk_sb = consts.tile([128, 8, HL], DT, name="wk_sb")
    wv_sb = consts.tile([128, 8, HL], DT, name="wv_sb")
    wo_sb = consts.tile([128, 2, D], DT, name="wo_sb")
    mtri_sb = consts.tile([128, 128], DT, name="mtri_sb")
    nc.sync.dma_start(out=wq_sb, in_=wqt)
    nc.sync.dma_start(out=wk_sb, in_=wkt)

    # --- persistent activations ---
    qT = [persist.tile([128, S], DT, name=f"qT{i}", tag=f"qT{i}") for i in range(2)]
    kT = [persist.tile([128, S], DT, name=f"kT{i}", tag=f"kT{i}") for i in range(2)]
    # v with an appended ones column per head: [token_tile, si, head, 65]
    v_sb = persist.tile([128, N_SI, 4, DK + 1], DT, name="v_sb", tag="v_sb")
    outTn = [
        persist.tile([128, S], DT, name=f"outTn{i}", tag=f"outTn{i}") for i in range(2)
    ]
    nc.vector.memset(v_sb[:, :, :, DK : DK + 1], 1.0)

    # --- input tiles, column-halved [d_tile, 1024-col half]; DMA issue order
    # == transfer order, so q/k half-0 stream in first (gates the first exp).
    def _ld_half(src, half, pref):
        ts = []
        for d in range(8):
            t = xt_pool.tile([128, 1024], DT, name=f"{pref}{half}_{d}", tag="xt")
            nc.sync.dma_start(
                out=t,
                in_=src[128 * d : 128 * d + 128, 1024 * half : 1024 * half + 1024],
            )
            ts.append(t)
        return ts

    q0 = _ld_half(qt, 0, "q")
    k0 = _ld_half(kt, 0, "k")
    v0 = _ld_half(vt, 0, "v")
    nc.sync.dma_start(out=wv_sb, in_=wvt)
    q1 = _ld_half(qt, 1, "q")
    k1 = _ld_half(kt, 1, "k")
    v1 = _ld_half(vt, 1, "v")
    nc.sync.dma_start(out=wo_sb, in_=wot)
    nc.sync.dma_start(out=mtri_sb, in_=mtri)

    # --- PE warm-up during the initial DMA wait; also preload the Exp
    # activation table so the first real exp doesn't pay the table load.
    junk = consts.tile([128, 512], DT, name="junk")
    dummy = consts.tile([1, 32], F32, name="dummy")
    nc.vector.memset(junk, 0.0)
    nc.scalar.activation(dummy, junk[0:1, 0:32], AF.Exp, scale=0.125)
    wps = psum.tile([128, 512], F32, name="warm", tag="P0")
    for _ in range(24):
        nc.tensor.matmul(wps, junk[:, 0:128], junk, start=True, stop=True)

    def _proj(w_sb, xts, hp, ptag, drain, dst, half):
        """dst[hp][:, half cols] = (W[hp's heads] @ X_half).T"""
        ps = psum.tile([128, 1024], F32, name=f"pj{half}_{hp}_{ptag}", tag=ptag)
        for d in range(8):
            for j in range(2):
                js = slice(512 * j, 512 * j + 512)
                nc.tensor.matmul(
                    ps[:, js],
                    w_sb[:, d, 128 * hp : 128 * hp + 128],
                    xts[d][:, js],
                    start=(d == 0),
                    stop=(d == 7),
                )
        cols = slice(1024 * half, 1024 * half + 1024)
        if drain == "scalar":
            nc.scalar.copy(dst[hp][:, cols], ps)
        else:
            nc.vector.tensor_copy(out=dst[hp][:, cols], in_=ps)

    def _vproj(xts, si0, ptag):
        """Project v for key tiles si0..si0+3 (one psum tile, 4 col groups)."""
        ps = psum.tile([128, 1024], F32, name=f"vps_{si0}", tag=ptag)
        for idx in range(4):
            ss = si0 + idx
            for d in range(8):
                nc.tensor.matmul(
                    ps[:, 256 * idx : 256 * idx + 256],
                    xts[d][:, 128 * (ss % 8) : 128 * (ss % 8) + 128],
                    wv_sb[:, d, :],
                    start=(d == 0),
                    stop=(d == 7),
                    skip_group_check=True,
                )
        nc.vector.tensor_copy(
            out=v_sb[:, si0 : si0 + 4, :, 0:DK],
            in_=ps.rearrange("p (s h d) -> p s h d", s=4, h=4),
        )

    # ---------------- attention machinery ----------------
    SC_TAGS = ["P0", "P1"]
    OT_TAGS = ["P2", "P3"]
    state = {"sc": 0, "ot": 0}

    def _scores(hp, ch, si, mask_engine="vector"):
        """Score tiles + exp + diag mask for both heads of pair hp.  The
        mask-mul is Scalar-paced (it reads the exp output), so on whichever
        engine it runs it parks in that engine's in-order queue until the exp
        lands; masks are routed (vector vs gpsimd) so they never queue ahead
        of the next pair's psum-release ops on DVE."""
        ch_lo = 1024 * ch
        t_min = 128 * si
        off = max(t_min - ch_lo, 0)
        banks = [tj for tj in (2 * ch, 2 * ch + 1) if 512 * tj + 512 > t_min]
        eng = nc.vector if mask_engine == "vector" else nc.gpsimd
        res = {}
        for h in (2 * hp, 2 * hp + 1):
            hr = 64 * (h % 2)
            scs = psum.tile(
                [128, 1024], F32, name=f"sc_{h}_{si}_{ch}", tag=SC_TAGS[state["sc"] % 2]
            )
            state["sc"] += 1
            for tj in banks:
                a = max(512 * tj, t_min)
                rel = slice(a - ch_lo, 512 * tj - ch_lo + 512)
                nc.tensor.matmul(
                    scs[:, rel],
                    kT[hp][hr : hr + 64, t_min : t_min + 128],
                    qT[hp][hr : hr + 64, a : 512 * tj + 512],
                    start=True,
                    stop=True,
                )
            att = attn_pool.tile([128, 1024], DT, name=f"at_{h}_{si}_{ch}", tag="at")
            nc.scalar.activation(
                att[:, off:1024], scs[:, off:1024], AF.Exp, scale=0.125
            )
            if mask_engine is not None and ch_lo <= t_min < ch_lo + 1024:
                eng.tensor_mul(
                    att[:, off : off + 128], att[:, off : off + 128], mtri_sb
                )
            res[h] = att
        return res

    def _mask(avatts, si, ch):
        """Deferred diag mask (DVE): emitted only after the preceding pair's
        psum-release ops so it never head-of-line-blocks them in the DVE
        queue (it is Scalar-paced: it waits on its exp)."""
        off = max(128 * si - 1024 * ch, 0)
        for h, att in avatts.items():
            nc.vector.tensor_mul(
                att[:, off : off + 128], att[:, off : off + 128], mtri_sb
            )

    def _emit_avs(hp, ch, ot, avsi, avatts):
        """attn@V for key tile avsi into the pair's ot accumulators."""
        ch_lo = 1024 * ch
        av_tmin = 128 * avsi
        for h in (2 * hp, 2 * hp + 1):
            for tj in (2 * ch, 2 * ch + 1):
                if 512 * tj + 512 <= av_tmin:
                    continue
                a = max(512 * tj, av_tmin)
                b = 512 * tj + 512
                nc.tensor.matmul(
                    ot[h][0:65, a - ch_lo : b - ch_lo],
                    v_sb[:, avsi, h, :],
                    avatts[h][:, a - ch_lo : b - ch_lo],
                    start=(avsi == 0),
                    stop=(avsi == 4 * tj + 3),
                    skip_group_check=True,
                )

    def _alloc_ot(hp, ch):
        return {
            h: psum.tile([128, 1024], F32, name=f"ot_{h}_{ch}", tag=OT_TAGS[h % 2])
            for h in (2 * hp, 2 * hp + 1)
        }

    def _normalize(hp, ch, ot, rel, mul_engine="gpsimd"):
        """outTn[.., rel cols of chunk ch] = ot rows / denominator row, for
        both heads of the pair.  The psum is released by just two DVE ops
        (raw copy + a 32-row transpose straight out of psum; rows 65:96 are
        junk and only the denominator row's transposed column is used).  The
        reciprocal runs on 32 DVE lanes; the broadcast and in-place scale of
        outTn run on gpsimd, off every critical path (gpsimd has multi-us
        op-turnaround latency, so it must never gate psum slots or the PE)."""
        ch_lo = 1024 * ch
        n = rel.stop - rel.start
        cols = slice(ch_lo + rel.start, ch_lo + rel.stop)
        for h in (2 * hp, 2 * hp + 1):
            hr = 64 * (h % 2)
            nc.vector.tensor_copy(out=outTn[hp][hr : hr + 64, cols], in_=ot[h][0:64, rel])
            dtt = bc_pool.tile([32, 1024], F32, name=f"dtt_{h}_{ch}", tag="dtt")
            nc.vector.transpose(dtt[:, 0:n], ot[h][64:96, rel])
            c0 = dtt.rearrange("p (b c) -> p b c", c=32)[:, 0 : n // 32, 0:1]
            nc.vector.reciprocal(c0, c0)
            dtr = bc_pool.tile([32, 1024], F32, name=f"dtr_{h}_{ch}", tag="dtr")
            nc.vector.transpose(dtr[:, 0:n], dtt[:, 0:n])
            # full-128-partition broadcast so the scale's two SB inputs share
            # a base partition (verifier NCC_IBIR297)
            bcb = bc_pool.tile([128, 1024], F32, name=f"bcb_{h}_{ch}", tag="bc")
            nc.gpsimd.partition_broadcast(bcb[:, 0:n], dtr[0:1, 0:n])
            nc.vector.tensor_mul(
                outTn[hp][hr : hr + 64, cols],
                outTn[hp][hr : hr + 64, cols],
                bcb[hr : hr + 64, 0:n],
            )

    def _outproj(tt, tags, drain):
        """out[t-tile, :] = outTn[:, t-tile].T @ Wo (both 128-row halves)."""
        ts = slice(128 * tt, 128 * tt + 128)
        key = "ot" if tags is OT_TAGS else "sc"
        ps = psum.tile([128, D], F32, name=f"op_{tt}", tag=tags[state[key] % 2])
        state[key] += 1
        for kk in range(2):
            for nj in range(2):
                js = slice(512 * nj, 512 * nj + 512)
                nc.tensor.matmul(
                    ps[:, js],
                    outTn[kk][:, ts],
                    wo_sb[:, kk, js],
                    start=(kk == 0),
                    stop=(kk == 1),
                )
        osb = out_pool.tile([128, D], DT, name=f"osb_{tt}", tag="osb")
        if drain == "scalar":
            nc.scalar.copy(osb, ps)
        else:
            nc.vector.tensor_copy(out=osb, in_=ps)
        nc.sync.dma_start(out=out[ts, :], in_=osb)

    # --- half-0 q/k projections (ScalarE drains them: it is idle pre-exp) ---
    _proj(wq_sb, q0, 0, "P0", "scalar", qT, 0)
    _proj(wq_sb, q0, 1, "P1", "scalar", qT, 0)
    _proj(wk_sb, k0, 0, "P2", "scalar", kT, 0)
    _proj(wk_sb, k0, 1, "P3", "scalar", kT, 0)

    # ---------------- chunk 0 (t in [0, 1024)) ----------------
    # All 32 score tiles stream through P0/P1 at exp pace; P2/P3 meanwhile
    # run vproj, the attn@V accumulators, and the pair-1 projections (pair-0's
    # slot right after its accumulator frees so chunk 1 can start stall-free).
    atts0 = {}
    for hp in range(2):
        for si in range(8):
            atts0[(hp, si)] = _scores(hp, 0, si, mask_engine=None)

    # pair-0 half-1 projections run DURING chunk-0 attention: their inputs
    # stream in by ~32us, the PE is mostly idle then, and P2/P3 have free
    # gens before the accumulators claim them — so chunk 1's first scores
    # depend on nothing slower than the chunk-0 exp drain itself.  vproj must
    # be emitted BEFORE kproj-h1 (k1's DMA waits on slots freed by vproj's
    # reads of v0 — the reverse order deadlocks), hence both vproj tiles on
    # P3 and both late projections on P2.
    _proj(wq_sb, q1, 0, "P2", "vector", qT, 1)
    _vproj(v0, 0, "P3")
    _vproj(v0, 4, "P3")
    _proj(wk_sb, k1, 0, "P2", "vector", kT, 1)
    for si in range(8):
        _mask(atts0[(0, si)], si, 0)

    ot = _alloc_ot(0, 0)
    for si in range(8):
        _emit_avs(0, 0, ot, si, atts0[(0, si)])
    _normalize(0, 0, ot, slice(0, 512))
    _normalize(0, 0, ot, slice(512, 1024))
    # first chunk-1 score tiles BEFORE pair-1's attn@V block: the exp stream
    # crosses the chunk boundary without waiting for it
    atts = {}
    for si in range(3):
        atts[si] = _scores(0, 1, si, mask_engine=None)
    for si in range(8):
        _mask(atts0[(1, si)], si, 0)
    ot = _alloc_ot(1, 0)
    for si in range(8):
        _emit_avs(1, 0, ot, si, atts0[(1, si)])
    _normalize(1, 0, ot, slice(0, 512))
    _normalize(1, 0, ot, slice(512, 1024))

    # ---------------- chunk 1 (t in [1024, 2048)) ----------------
    # pair 0: scores interleaved with pair-1 projections and the v half-1
    # projection; attn@V trails (its accumulators wait on those P2/P3 gens).
    # All P2/P3 utility tiles must be allocated BEFORE this chunk's
    # accumulators (a later same-tag allocation waits on the accumulator's
    # release and deadlocks against the attn@V stream).
    ot = None
    avn = 0

    def _avs_upto(hp, upto):
        nonlocal avn
        while avn < upto:
            _emit_avs(hp, 1, ot, avn, atts.pop(avn))
            avn += 1

    # The pair-1 projections are pushed late enough in the PE stream that the
    # P2/P3 gens they wait on (chunk-0 accumulator releases) have already
    # freed by the time the in-order PE queue reaches them — an early emission
    # would head-of-line-block every later score matmul on that wait.
    for si in range(3, 16):
        atts[si] = _scores(0, 1, si, mask_engine=None)
        if si >= 8:
            _mask(atts[si], si, 1)
        if si == 6:
            _proj(wq_sb, q1, 1, "P2", "vector", qT, 1)
        elif si == 8:
            _proj(wk_sb, k1, 1, "P3", "vector", kT, 1)
        elif si == 9:
            _vproj(v1, 8, "P2")
        elif si == 10:
            _vproj(v1, 12, "P3")
            ot = _alloc_ot(0, 1)
        elif si == 11:
            _avs_upto(0, 4)
        elif si == 12:
            _avs_upto(0, 8)
        elif si >= 13:
            _avs_upto(0, si - 1)
        if si == 13:
            _avs_upto(0, 12)
            _normalize(0, 1, ot, slice(0, 512))
    _avs_upto(0, 16)
    _normalize(0, 1, ot, slice(512, 1024))

    # pair 1: scores interleaved with the chunk-0 output projection (riding
    # the P2/P3 slots freed by pair-0's normalize) and a lagged attn@V
    # stream (its accumulators only free up after the chunk-0 outproj).
    atts = {}
    ot = None
    avn = 0
    for si in range(16):
        atts[si] = _scores(1, 1, si, mask_engine=None)
        if si >= 8:
            _mask(atts[si], si, 1)
        if 1 <= si <= 8:
            _outproj(si - 1, OT_TAGS, "vector")
        if si == 8:
            ot = _alloc_ot(1, 1)
            _avs_upto(1, 2)
        elif si >= 9:
            _avs_upto(1, 2 * (si - 7))
        if si == 13:
            _normalize(1, 1, ot, slice(0, 512))
    _avs_upto(1, 16)
    # final normalize's scale on DVE: gpsimd's op latency would land on the
    # tail, and DVE is idle by now
    _normalize(1, 1, ot, slice(512, 1024), mul_engine="vector")

    # chunk-1 output projection: P0/P1 rotation behind the last score tiles,
    # drained on ScalarE (idle once the exps are done).
    for tt in range(8, 16):
        _outproj(tt, SC_TAGS, "scalar" if tt % 2 == 0 else "vector")

    for pool in (psum, out_pool, bc_pool, attn_pool, xt_pool, persist, consts):
        pool.release()


def _get_nc():
    global _CACHED_NC
    if _CACHED_NC is None:
        nc = bacc.Bacc("TRN2", target_bir_lowering=False, debug=False)
        with tile.TileContext(nc) as tc:
            _build_kernel(tc)
        nc.compile()
        _CACHED_NC = nc
    return _CACHED_NC


def kernel(Q, K, V, mask, Wq, Wk, Wv, Wo, bo):
    global LAST_EXEC_NS
    nc = _get_nc()
    mtri = np.triu(np.ones((128, 128), dtype=np.float32))
    in_maps = []
    for c in range(8):
        b, hg = c // 4, c % 4
        rs = slice(HL * hg, HL * hg + HL)
        in_maps.append(
            {
                "qt": _prep(np.asarray(Q, np.float32)[b].T),
                "kt": _prep(np.asarray(K, np.float32)[b].T),
                "vt": _prep(np.asarray(V, np.float32)[b].T),
                "wqt": _prep_w(np.asarray(Wq, np.float32)[rs].T),
                "wkt": _prep_w(np.asarray(Wk, np.float32)[rs].T),
                "wvt": _prep_w(np.asarray(Wv, np.float32)[rs].T),
                "wot": _prep_w(np.asarray(Wo, np.float32)[:, rs].T),
                "mtri": _prep(mtri),
            }
        )
    trace = os.environ.get("BASS_MHA_TRACE", "") == "1"
    res = run_bass_kernel_spmd(nc, in_maps, core_ids=list(range(8)), trace=trace)
    LAST_EXEC_NS = res.exec_time_ns
    outs = [np.asarray(res.results[c]["out"], np.float32) for c in range(8)]
    bo = np.asarray(bo, np.float32)
    full = np.stack(
        [
            outs[0] + outs[1] + outs[2] + outs[3] + bo,
            outs[4] + outs[5] + outs[6] + outs[7] + bo,
        ]
    ).astype(np.float32)
    return full


# revision 26
# speedup vs baseline: 1.1390x; 1.1390x over previous
"""Multi-head causal attention (B=2, S=2048, D=1024, H=16) on 8 trn2 cores.

Sharding (Megatron TP over batch*heads): core c handles batch c//4 and the
4 heads 4*(c%4)..4*(c%4)+3.  Wq/Wk/Wv are column-sharded (each core gets the
256 rows of W* for its heads), Wo is row-sharded; the host sums the 4 partial
outputs per batch (the tensor-parallel all-reduce) and adds bo.

The schedule is built around the ScalarE exp stream (the bottleneck, ~110us
of activations): projections are column-halved so the first score tiles exist
as soon as the first halves of q/k stream in, attention runs t-chunk-outer /
head-pair-inner, and psum->sbuf drains run on DVE/gpsimd (except pre-exp and
post-exp windows where ScalarE is idle anyway).  PSUM is four 2-bank slots:
P0/P1 rotate score tiles, P2/P3 hold attn@V accumulators and v/late
projection groups between accumulations.  Softmax normalization keeps the
psum critical path short (raw copies only) and does the reciprocal via a
32-lane transpose dance (DVE reciprocal costs ~8ns/elem per partition);
the final scale runs on gpsimd off every critical engine.
"""

import os
import sys

sys.path.insert(0, "/opt/trn_rl_repo")

import ml_dtypes
import numpy as np

import concourse.bass as bass  # noqa: F401
import concourse.bass2jax as bass2jax
import concourse.tile as tile
from concourse import bacc, mybir
from concourse.bass_utils import run_bass_kernel_spmd

# Optional NEFF tee for local profiling (active only when the env var is set).
_orig_rename = bass2jax.rename_neff_tensors_and_patch_header


def _tee_rename(neff_path, mapping):
    data = _orig_rename(neff_path, mapping)
    tee = os.environ.get("BASS_MHA_NEFF_TEE")
    if tee:
        try:
            with open(tee, "wb") as f:
                f.write(data)
        except OSError:
            pass
    return data


bass2jax.rename_neff_tensors_and_patch_header = _tee_rename

F32 = mybir.dt.float32
AF = mybir.ActivationFunctionType

S = 2048  # sequence length
D = 1024  # model dim
HL = 256  # local head width (4 heads x 64)
DK = 64  # head dim
N_SI = S // 128  # 16 key tiles

MODE = os.environ.get("BASS_MHA_DTYPE", "bf16")  # bf16 | f32r | f32
if MODE == "bf16":
    DT = mybir.dt.bfloat16
    NPDT = ml_dtypes.bfloat16
elif MODE == "f32r":
    DT = mybir.dt.float32r
    NPDT = np.float32
else:
    DT = F32
    NPDT = np.float32

LAST_EXEC_NS = None
_CACHED_NC = None


def _prep(a):
    """Cast a host array to the kernel's compute dtype (with fp32r rounding
    matching the compiler's fp32_to_fp32r when in f32r mode)."""
    a = np.ascontiguousarray(np.asarray(a, np.float32))
    if MODE == "bf16":
        return a.astype(ml_dtypes.bfloat16)
    if MODE == "f32r":
        bits = a.view(np.uint32).astype(np.uint64)
        rounded = (bits + 0x7FF + ((bits >> 12) & 1)) & 0xFFFFF000
        return rounded.astype(np.uint32).view(np.float32).reshape(a.shape)
    return a


def _prep_w(w):
    """[n*128, c] weight -> [128, n, c] partition-major layout (4KB DMA rows)."""
    return _prep(
        np.ascontiguousarray(w.reshape(-1, 128, w.shape[-1]).transpose(1, 0, 2))
    )


def _build_kernel(tc):
    nc = tc.nc
    qt = nc.dram_tensor("qt", [D, S], DT, kind="ExternalInput").ap()
    kt = nc.dram_tensor("kt", [D, S], DT, kind="ExternalInput").ap()
    vt = nc.dram_tensor("vt", [D, S], DT, kind="ExternalInput").ap()
    wqt = nc.dram_tensor("wqt", [128, 8, HL], DT, kind="ExternalInput").ap()
    wkt = nc.dram_tensor("wkt", [128, 8, HL], DT, kind="ExternalInput").ap()
    wvt = nc.dram_tensor("wvt", [128, 8, HL], DT, kind="ExternalInput").ap()
    wot = nc.dram_tensor("wot", [128, 2, D], DT, kind="ExternalInput").ap()
    mtri = nc.dram_tensor("mtri", [128, 128], DT, kind="ExternalInput").ap()
    out = nc.dram_tensor("out", [S, D], DT, kind="ExternalOutput").ap()

    consts = tc.alloc_tile_pool(name="consts", bufs=1)
    persist = tc.alloc_tile_pool(name="persist", bufs=1)
    xt_pool = tc.alloc_tile_pool(name="xt", bufs=16)
    attn_pool = tc.alloc_tile_pool(name="attn", bufs=22)
    bc_pool = tc.alloc_tile_pool(name="bc", bufs=4)
    out_pool = tc.alloc_tile_pool(name="outsb", bufs=4)
    psum = tc.alloc_tile_pool(name="ps", bufs=1, space="PSUM")

    # --- weights in SBUF (already partition-major in dram: 4KB rows) ---
    wq_sb = consts.tile([128, 8, HL], DT, name="wq_sb")
    wk_sb = consts.tile([128, 8, HL], DT, name="wk_sb")
    wv_sb = consts.tile([128, 8, HL], DT, name="wv_sb")
    wo_sb = consts.tile([128, 2, D], DT, name="wo_sb")
    mtri_sb = consts.tile([128, 128], DT, name="mtri_sb")
    nc.sync.dma_start(out=wq_sb, in_=wqt)
    nc.sync.dma_start(out=wk_sb, in_=wkt)

    # --- persistent activations ---
    qT = [persist.tile([128, S], DT, name=f"qT{i}", tag=f"qT{i}") for i in range(2)]
    kT = [persist.tile([128, S], DT, name=f"kT{i}", tag=f"kT{i}") for i in range(2)]
    # v with an appended ones column per head: [token_tile, si, head, 65]
    v_sb = persist.tile([128, N_SI, 4, DK + 1], DT, name="v_sb", tag="v_sb")
    outTn = [
        persist.tile([128, S], DT, name=f"outTn{i}", tag=f"outTn{i}") for i in range(2)
    ]
    nc.vector.memset(v_sb[:, :, :, DK : DK + 1], 1.0)

    # --- input tiles, column-halved [d_tile, 1024-col half]; DMA issue order
    # == transfer order, so q/k half-0 stream in first (gates the first exp).
    def _ld_half(src, half, pref):
        ts = []
        for d in range(8):
            t = xt_pool.tile([128, 1024], DT, name=f"{pref}{half}_{d}", tag="xt")
            nc.sync.dma_start(
                out=t,
                in_=src[128 * d : 128 * d + 128, 1024 * half : 1024 * half + 1024],
            )
            ts.append(t)
        return ts

    q0 = _ld_half(qt, 0, "q")
    k0 = _ld_half(kt, 0, "k")
    v0 = _ld_half(vt, 0, "v")
    nc.sync.dma_start(out=wv_sb, in_=wvt)
    q1 = _ld_half(qt, 1, "q")
    k1 = _ld_half(kt, 1, "k")
    v1 = _ld_half(vt, 1, "v")
    nc.sync.dma_start(out=wo_sb, in_=wot)
    nc.sync.dma_start(out=mtri_sb, in_=mtri)

    # --- PE warm-up during the initial DMA wait; also preload the Exp
    # activation table so the first real exp doesn't pay the table load.
    junk = consts.tile([128, 512], DT, name="junk")
    dummy = consts.tile([1, 32], F32, name="dummy")
    nc.vector.memset(junk, 0.0)
    nc.scalar.activation(dummy, junk[0:1, 0:32], AF.Exp, scale=0.125)
    wps = psum.tile([128, 512], F32, name="warm", tag="P0")
    for _ in range(24):
        nc.tensor.matmul(wps, junk[:, 0:128], junk, start=True, stop=True)

    def _proj(w_sb, xts, hp, ptag, drain, dst, half):
        """dst[hp][:, half cols] = (W[hp's heads] @ X_half).T"""
        ps = psum.tile([128, 1024], F32, name=f"pj{half}_{hp}_{ptag}", tag=ptag)
        for d in range(8):
            for j in range(2):
                js = slice(512 * j, 512 * j + 512)
                nc.tensor.matmul(
                    ps[:, js],
                    w_sb[:, d, 128 * hp : 128 * hp + 128],
                    xts[d][:, js],
                    start=(d == 0),
                    stop=(d == 7),
                )
        cols = slice(1024 * half, 1024 * half + 1024)
        if drain == "scalar":
            nc.scalar.copy(dst[hp][:, cols], ps)
        else:
            nc.vector.tensor_copy(out=dst[hp][:, cols], in_=ps)

    def _vproj(xts, si0, ptag):
        """Project v for key tiles si0..si0+3 (one psum tile, 4 col groups)."""
        ps = psum.tile([128, 1024], F32, name=f"vps_{si0}", tag=ptag)
        for idx in range(4):
            ss = si0 + idx
            for d in range(8):
                nc.tensor.matmul(
                    ps[:, 256 * idx : 256 * idx + 256],
                    xts[d][:, 128 * (ss % 8) : 128 * (ss % 8) + 128],
                    wv_sb[:, d, :],
                    start=(d == 0),
                    stop=(d == 7),
                    skip_group_check=True,
                )
        nc.vector.tensor_copy(
            out=v_sb[:, si0 : si0 + 4, :, 0:DK],
            in_=ps.rearrange("p (s h d) -> p s h d", s=4, h=4),
        )

    # ---------------- attention machinery ----------------
    SC_TAGS = ["P0", "P1"]
    OT_TAGS = ["P2", "P3"]
    state = {"sc": 0, "ot": 0}

    def _scores(hp, ch, si, mask_engine="vector"):
        """Score tiles + exp + diag mask for both heads of pair hp.  The
        mask-mul is Scalar-paced (it reads the exp output), so on whichever
        engine it runs it parks in that engine's in-order queue until the exp
        lands; masks are routed (vector vs gpsimd) so they never queue ahead
        of the next pair's psum-release ops on DVE."""
        ch_lo = 1024 * ch
        t_min = 128 * si
        off = max(t_min - ch_lo, 0)
        banks = [tj for tj in (2 * ch, 2 * ch + 1) if 512 * tj + 512 > t_min]
        eng = nc.vector if mask_engine == "vector" else nc.gpsimd
        res = {}
        for h in (2 * hp, 2 * hp + 1):
            hr = 64 * (h % 2)
            scs = psum.tile(
                [128, 1024], F32, name=f"sc_{h}_{si}_{ch}", tag=SC_TAGS[state["sc"] % 2]
            )
            state["sc"] += 1
            for tj in banks:
                a = max(512 * tj, t_min)
                rel = slice(a - ch_lo, 512 * tj - ch_lo + 512)
                nc.tensor.matmul(
                    scs[:, rel],
                    kT[hp][hr : hr + 64, t_min : t_min + 128],
                    qT[hp][hr : hr + 64, a : 512 * tj + 512],
                    start=True,
                    stop=True,
                )
            att = attn_pool.tile([128, 1024], DT, name=f"at_{h}_{si}_{ch}", tag="at")
            nc.scalar.activation(
                att[:, off:1024], scs[:, off:1024], AF.Exp, scale=0.125
            )
            if mask_engine is not None and ch_lo <= t_min < ch_lo + 1024:
                eng.tensor_mul(
                    att[:, off : off + 128], att[:, off : off + 128], mtri_sb
                )
            res[h] = att
        return res

    def _mask(avatts, si, ch):
        """Deferred diag mask (DVE): emitted only after the preceding pair's
        psum-release ops so it never head-of-line-blocks them in the DVE
        queue (it is Scalar-paced: it waits on its exp)."""
        off = max(128 * si - 1024 * ch, 0)
        for h, att in avatts.items():
            nc.vector.tensor_mul(
                att[:, off : off + 128], att[:, off : off + 128], mtri_sb
            )

    def _emit_avs(hp, ch, ot, avsi, avatts):
        """attn@V for key tile avsi into the pair's ot accumulators."""
        ch_lo = 1024 * ch
        av_tmin = 128 * avsi
        for h in (2 * hp, 2 * hp + 1):
            for tj in (2 * ch, 2 * ch + 1):
                if 512 * tj + 512 <= av_tmin:
                    continue
                a = max(512 * tj, av_tmin)
                b = 512 * tj + 512
                nc.tensor.matmul(
                    ot[h][0:65, a - ch_lo : b - ch_lo],
                    v_sb[:, avsi, h, :],
                    avatts[h][:, a - ch_lo : b - ch_lo],
                    start=(avsi == 0),
                    stop=(avsi == 4 * tj + 3),
                    skip_group_check=True,
                )

    def _alloc_ot(hp, ch):
        return {
            h: psum.tile([128, 1024], F32, name=f"ot_{h}_{ch}", tag=OT_TAGS[h % 2])
            for h in (2 * hp, 2 * hp + 1)
        }

    def _normalize(hp, ch, ot, rel, mul_engine="gpsimd"):
        """outTn[.., rel cols of chunk ch] = ot rows / denominator row, for
        both heads of the pair.  The psum is released by just two DVE ops
        (raw copy + a 32-row transpose straight out of psum; rows 65:96 are
        junk and only the denominator row's transposed column is used).  The
        reciprocal runs on 32 DVE lanes; the broadcast and in-place scale of
        outTn run on gpsimd, off every critical path (gpsimd has multi-us
        op-turnaround latency, so it must never gate psum slots or the PE)."""
        ch_lo = 1024 * ch
        n = rel.stop - rel.start
        cols = slice(ch_lo + rel.start, ch_lo + rel.stop)
        for h in (2 * hp, 2 * hp + 1):
            hr = 64 * (h % 2)
            nc.vector.tensor_copy(out=outTn[hp][hr : hr + 64, cols], in_=ot[h][0:64, rel])
            dtt = bc_pool.tile([32, 1024], F32, name=f"dtt_{h}_{ch}", tag="dtt")
            nc.vector.transpose(dtt[:, 0:n], ot[h][64:96, rel])
            c0 = dtt.rearrange("p (b c) -> p b c", c=32)[:, 0 : n // 32, 0:1]
            nc.vector.reciprocal(c0, c0)
            dtr = bc_pool.tile([32, 1024], F32, name=f"dtr_{h}_{ch}", tag="dtr")
            nc.vector.transpose(dtr[:, 0:n], dtt[:, 0:n])
            # full-128-partition broadcast so the scale's two SB inputs share
            # a base partition (verifier NCC_IBIR297)
            bcb = bc_pool.tile([128, 1024], F32, name=f"bcb_{h}_{ch}", tag="bc")
            nc.gpsimd.partition_broadcast(bcb[:, 0:n], dtr[0:1, 0:n])
            nc.vector.tensor_mul(
                outTn[hp][hr : hr + 64, cols],
                outTn[hp][hr : hr + 64, cols],
                bcb[hr : hr + 64, 0:n],
            )

    def _outproj(tt, tags, drain):
        """out[t-tile, :] = outTn[:, t-tile].T @ Wo (both 128-row halves)."""
        ts = slice(128 * tt, 128 * tt + 128)
        key = "ot" if tags is OT_TAGS else "sc"
        ps = psum.tile([128, D], F32, name=f"op_{tt}", tag=tags[state[key] % 2])
        state[key] += 1
        for kk in range(2):
            for nj in range(2):
                js = slice(512 * nj, 512 * nj + 512)
                nc.tensor.matmul(
                    ps[:, js],
                    outTn[kk][:, ts],
                    wo_sb[:, kk, js],
                    start=(kk == 0),
                    stop=(kk == 1),
                )
        osb = out_pool.tile([128, D], DT, name=f"osb_{tt}", tag="osb")
        if drain == "scalar":
            nc.scalar.copy(osb, ps)
        else:
            nc.vector.tensor_copy(out=osb, in_=ps)
        nc.sync.dma_start(out=out[ts, :], in_=osb)

    # --- half-0 q/k projections (ScalarE drains them: it is idle pre-exp) ---
    _proj(wq_sb, q0, 0, "P0", "scalar", qT, 0)
    _proj(wq_sb, q0, 1, "P1", "scalar", qT, 0)
    _proj(wk_sb, k0, 0, "P2", "scalar", kT, 0)
    _proj(wk_sb, k0, 1, "P3", "scalar", kT, 0)

    # ---------------- chunk 0 (t in [0, 1024)) ----------------
    # All 32 score tiles stream through P0/P1 at exp pace; P2/P3 meanwhile
    # run vproj, the attn@V accumulators, and the pair-1 projections (pair-0's
    # slot right after its accumulator frees so chunk 1 can start stall-free).
    atts0 = {}
    for hp in range(2):
        for si in range(8):
            atts0[(hp, si)] = _scores(hp, 0, si, mask_engine=None)

    for si in range(8):
        _mask(atts0[(0, si)], si, 0)
    _vproj(v0, 0, "P2")
    _vproj(v0, 4, "P3")

    ot = _alloc_ot(0, 0)
    for si in range(8):
        _emit_avs(0, 0, ot, si, atts0[(0, si)])
    _normalize(0, 0, ot, slice(0, 512))
    _normalize(0, 0, ot, slice(512, 1024))
    # pair-0 half-1 projections: claim the P2/P3 gens freed by pair-0's
    # chunk-0 accumulators so qT/kT half-1 exist before the chunk-0 exps end
    _proj(wq_sb, q1, 0, "P2", "vector", qT, 1)
    # first chunk-1 score tiles immediately after their only producer
    # (qproj half-1 pair-0): queued ahead of kproj/avs-hp1/normalize on the
    # PE so the exp stream crosses the chunk boundary ~10us earlier (they
    # use keys < 1024, so kT half-0 suffices)
    atts = {}
    for si in range(3):
        atts[si] = _scores(0, 1, si, mask_engine=None)
    _proj(wk_sb, k1, 0, "P3", "vector", kT, 1)
    for si in range(8):
        _mask(atts0[(1, si)], si, 0)
    ot = _alloc_ot(1, 0)
    for si in range(8):
        _emit_avs(1, 0, ot, si, atts0[(1, si)])
    _normalize(1, 0, ot, slice(0, 512))
    _normalize(1, 0, ot, slice(512, 1024))

    # ---------------- chunk 1 (t in [1024, 2048)) ----------------
    # pair 0: scores interleaved with pair-1 projections and the v half-1
    # projection; attn@V trails (its accumulators wait on those P2/P3 gens).
    # All P2/P3 utility tiles must be allocated BEFORE this chunk's
    # accumulators (a later same-tag allocation waits on the accumulator's
    # release and deadlocks against the attn@V stream).
    ot = None
    avn = 0

    def _avs_upto(hp, upto):
        nonlocal avn
        while avn < upto:
            _emit_avs(hp, 1, ot, avn, atts.pop(avn))
            avn += 1

    # The pair-1 projections are pushed late enough in the PE stream that the
    # P2/P3 gens they wait on (chunk-0 accumulator releases) have already
    # freed by the time the in-order PE queue reaches them — an early emission
    # would head-of-line-block every later score matmul on that wait.
    for si in range(3, 16):
        atts[si] = _scores(0, 1, si, mask_engine=None)
        if si >= 8:
            _mask(atts[si], si, 1)
        if si == 6:
            _proj(wq_sb, q1, 1, "P2", "vector", qT, 1)
        elif si == 8:
            _proj(wk_sb, k1, 1, "P3", "vector", kT, 1)
        elif si == 9:
            _vproj(v1, 8, "P2")
        elif si == 10:
            _vproj(v1, 12, "P3")
            ot = _alloc_ot(0, 1)
        elif si == 11:
            _avs_upto(0, 4)
        elif si == 12:
            _avs_upto(0, 8)
        elif si >= 13:
            _avs_upto(0, si - 1)
        if si == 13:
            _avs_upto(0, 12)
            _normalize(0, 1, ot, slice(0, 512))
    _avs_upto(0, 16)
    _normalize(0, 1, ot, slice(512, 1024))

    # pair 1: scores interleaved with the chunk-0 output projection (riding
    # the P2/P3 slots freed by pair-0's normalize) and a lagged attn@V
    # stream (its accumulators only free up after the chunk-0 outproj).
    atts = {}
    ot = None
    avn = 0
    for si in range(16):
        atts[si] = _scores(1, 1, si, mask_engine=None)
        if si >= 8:
            _mask(atts[si], si, 1)
        if 1 <= si <= 8:
            _outproj(si - 1, OT_TAGS, "vector")
        if si == 8:
            ot = _alloc_ot(1, 1)
            _avs_upto(1, 2)
        elif si >= 9:
            _avs_upto(1, 2 * (si - 7))
        if si == 13:
            _normalize(1, 1, ot, slice(0, 512))
    _avs_upto(1, 16)
    # final normalize's scale on DVE: gpsimd's op latency would land on the
    # tail, and DVE is idle by now
    _normalize(1, 1, ot, slice(512, 1024), mul_engine="vector")

    # chunk-1 output projection: P0/P1 rotation behind the last score tiles,
    # drained on ScalarE (idle once the exps are done).
    for tt in range(8, 16):
        _outproj(tt, SC_TAGS, "scalar" if tt % 2 == 0 else "vector")

    for pool in (psum, out_pool, bc_pool, attn_pool, xt_pool, persist, consts):
        pool.release()


def _get_nc():
    global _CACHED_NC
    if _CACHED_NC is None:
        nc = bacc.Bacc("TRN2", target_bir_lowering=False, debug=False)
        with tile.TileContext(nc) as tc:
            _build_kernel(tc)
        nc.compile()
        _CACHED_NC = nc
    return _CACHED_NC


def kernel(Q, K, V, mask, Wq, Wk, Wv, Wo, bo):
    global LAST_EXEC_NS
    nc = _get_nc()
    mtri = np.triu(np.ones((128, 128), dtype=np.float32))
    in_maps = []
    for c in range(8):
        b, hg = c // 4, c % 4
        rs = slice(HL * hg, HL * hg + HL)
        in_maps.append(
            {
                "qt": _prep(np.asarray(Q, np.float32)[b].T),
                "kt": _prep(np.asarray(K, np.float32)[b].T),
                "vt": _prep(np.asarray(V, np.float32)[b].T),
                "wqt": _prep_w(np.asarray(Wq, np.float32)[rs].T),
                "wkt": _prep_w(np.asarray(Wk, np.float32)[rs].T),
                "wvt": _prep_w(np.asarray(Wv, np.float32)[rs].T),
                "wot": _prep_w(np.asarray(Wo, np.float32)[:, rs].T),
                "mtri": _prep(mtri),
            }
        )
    trace = os.environ.get("BASS_MHA_TRACE", "") == "1"
    res = run_bass_kernel_spmd(nc, in_maps, core_ids=list(range(8)), trace=trace)
    LAST_EXEC_NS = res.exec_time_ns
    outs = [np.asarray(res.results[c]["out"], np.float32) for c in range(8)]
    bo = np.asarray(bo, np.float32)
    full = np.stack(
        [
            outs[0] + outs[1] + outs[2] + outs[3] + bo,
            outs[4] + outs[5] + outs[6] + outs[7] + bo,
        ]
    ).astype(np.float32)
    return full


# revision 27
# speedup vs baseline: 1.2228x; 1.0736x over previous
"""Multi-head causal attention (B=2, S=2048, D=1024, H=16) on 8 trn2 cores.

Sharding (Megatron TP over batch*heads): core c handles batch c//4 and the
4 heads 4*(c%4)..4*(c%4)+3.  Wq/Wk/Wv are column-sharded (each core gets the
256 rows of W* for its heads), Wo is row-sharded; the host sums the 4 partial
outputs per batch (the tensor-parallel all-reduce) and adds bo.

The schedule is built around the ScalarE exp stream (the bottleneck, ~110us
of activations): projections are column-halved so the first score tiles exist
as soon as the first halves of q/k stream in, attention runs t-chunk-outer /
head-pair-inner, and psum->sbuf drains run on DVE/gpsimd (except pre-exp and
post-exp windows where ScalarE is idle anyway).  PSUM is four 2-bank slots:
P0/P1 rotate score tiles, P2/P3 hold attn@V accumulators and v/late
projection groups between accumulations.  Softmax normalization keeps the
psum critical path short (raw copies only) and does the reciprocal via a
32-lane transpose dance (DVE reciprocal costs ~8ns/elem per partition);
the final scale runs on gpsimd off every critical engine.
"""

import os
import sys

sys.path.insert(0, "/opt/trn_rl_repo")

import ml_dtypes
import numpy as np

import concourse.bass as bass  # noqa: F401
import concourse.bass2jax as bass2jax
import concourse.tile as tile
from concourse import bacc, mybir
from concourse.bass_utils import run_bass_kernel_spmd

# Optional NEFF tee for local profiling (active only when the env var is set).
_orig_rename = bass2jax.rename_neff_tensors_and_patch_header


def _tee_rename(neff_path, mapping):
    data = _orig_rename(neff_path, mapping)
    tee = os.environ.get("BASS_MHA_NEFF_TEE")
    if tee:
        try:
            with open(tee, "wb") as f:
                f.write(data)
        except OSError:
            pass
    return data


bass2jax.rename_neff_tensors_and_patch_header = _tee_rename

F32 = mybir.dt.float32
AF = mybir.ActivationFunctionType

S = 2048  # sequence length
D = 1024  # model dim
HL = 256  # local head width (4 heads x 64)
DK = 64  # head dim
N_SI = S // 128  # 16 key tiles

MODE = os.environ.get("BASS_MHA_DTYPE", "bf16")  # bf16 | f32r | f32
if MODE == "bf16":
    DT = mybir.dt.bfloat16
    NPDT = ml_dtypes.bfloat16
elif MODE == "f32r":
    DT = mybir.dt.float32r
    NPDT = np.float32
else:
    DT = F32
    NPDT = np.float32

LAST_EXEC_NS = None
_CACHED_NC = None


def _prep(a):
    """Cast a host array to the kernel's compute dtype (with fp32r rounding
    matching the compiler's fp32_to_fp32r when in f32r mode)."""
    a = np.ascontiguousarray(np.asarray(a, np.float32))
    if MODE == "bf16":
        return a.astype(ml_dtypes.bfloat16)
    if MODE == "f32r":
        bits = a.view(np.uint32).astype(np.uint64)
        rounded = (bits + 0x7FF + ((bits >> 12) & 1)) & 0xFFFFF000
        return rounded.astype(np.uint32).view(np.float32).reshape(a.shape)
    return a


def _prep_w(w):
    """[n*128, c] weight -> [128, n, c] partition-major layout (4KB DMA rows)."""
    return _prep(
        np.ascontiguousarray(w.reshape(-1, 128, w.shape[-1]).transpose(1, 0, 2))
    )


def _build_kernel(tc):
    nc = tc.nc
    qt = nc.dram_tensor("qt", [D, S], DT, kind="ExternalInput").ap()
    kt = nc.dram_tensor("kt", [D, S], DT, kind="ExternalInput").ap()
    vt = nc.dram_tensor("vt", [D, S], DT, kind="ExternalInput").ap()
    wqt = nc.dram_tensor("wqt", [128, 8, HL], DT, kind="ExternalInput").ap()
    wkt = nc.dram_tensor("wkt", [128, 8, HL], DT, kind="ExternalInput").ap()
    wvt = nc.dram_tensor("wvt", [128, 8, HL], DT, kind="ExternalInput").ap()
    wot = nc.dram_tensor("wot", [128, 2, D], DT, kind="ExternalInput").ap()
    mtri = nc.dram_tensor("mtri", [128, 128], DT, kind="ExternalInput").ap()
    out = nc.dram_tensor("out", [S, D], DT, kind="ExternalOutput").ap()

    consts = tc.alloc_tile_pool(name="consts", bufs=1)
    persist = tc.alloc_tile_pool(name="persist", bufs=1)
    xt_pool = tc.alloc_tile_pool(name="xt", bufs=16)
    attn_pool = tc.alloc_tile_pool(name="attn", bufs=22)
    bc_pool = tc.alloc_tile_pool(name="bc", bufs=4)
    out_pool = tc.alloc_tile_pool(name="outsb", bufs=4)
    psum = tc.alloc_tile_pool(name="ps", bufs=1, space="PSUM")

    # --- weights in SBUF (already partition-major in dram: 4KB rows) ---
    wq_sb = consts.tile([128, 8, HL], DT, name="wq_sb")
    wk_sb = consts.tile([128, 8, HL], DT, name="wk_sb")
    wv_sb = consts.tile([128, 8, HL], DT, name="wv_sb")
    wo_sb = consts.tile([128, 2, D], DT, name="wo_sb")
    mtri_sb = consts.tile([128, 128], DT, name="mtri_sb")
    nc.sync.dma_start(out=wq_sb, in_=wqt)
    nc.sync.dma_start(out=wk_sb, in_=wkt)

    # --- persistent activations ---
    qT = [persist.tile([128, S], DT, name=f"qT{i}", tag=f"qT{i}") for i in range(2)]
    kT = [persist.tile([128, S], DT, name=f"kT{i}", tag=f"kT{i}") for i in range(2)]
    # v with an appended ones column per head: [token_tile, si, head, 65]
    v_sb = persist.tile([128, N_SI, 4, DK + 1], DT, name="v_sb", tag="v_sb")
    outTn = [
        persist.tile([128, S], DT, name=f"outTn{i}", tag=f"outTn{i}") for i in range(2)
    ]
    nc.vector.memset(v_sb[:, :, :, DK : DK + 1], 1.0)

    # --- input tiles, column-halved [d_tile, 1024-col half]; DMA issue order
    # == transfer order, so q/k half-0 stream in first (gates the first exp).
    def _ld_half(src, half, pref):
        ts = []
        for d in range(8):
            t = xt_pool.tile([128, 1024], DT, name=f"{pref}{half}_{d}", tag="xt")
            nc.sync.dma_start(
                out=t,
                in_=src[128 * d : 128 * d + 128, 1024 * half : 1024 * half + 1024],
            )
            ts.append(t)
        return ts

    q0 = _ld_half(qt, 0, "q")
    k0 = _ld_half(kt, 0, "k")
    v0 = _ld_half(vt, 0, "v")
    nc.sync.dma_start(out=wv_sb, in_=wvt)
    q1 = _ld_half(qt, 1, "q")
    k1 = _ld_half(kt, 1, "k")
    v1 = _ld_half(vt, 1, "v")
    nc.sync.dma_start(out=wo_sb, in_=wot)
    nc.sync.dma_start(out=mtri_sb, in_=mtri)

    # --- PE warm-up during the initial DMA wait; also preload the Exp
    # activation table so the first real exp doesn't pay the table load.
    junk = consts.tile([128, 512], DT, name="junk")
    dummy = consts.tile([1, 32], F32, name="dummy")
    nc.vector.memset(junk, 0.0)
    nc.scalar.activation(dummy, junk[0:1, 0:32], AF.Exp, scale=0.125)
    wps = psum.tile([128, 512], F32, name="warm", tag="P0")
    for _ in range(24):
        nc.tensor.matmul(wps, junk[:, 0:128], junk, start=True, stop=True)

    def _proj(w_sb, xts, hp, ptag, drain, dst, half):
        """dst[hp][:, half cols] = (W[hp's heads] @ X_half).T"""
        ps = psum.tile([128, 1024], F32, name=f"pj{half}_{hp}_{ptag}", tag=ptag)
        for d in range(8):
            for j in range(2):
                js = slice(512 * j, 512 * j + 512)
                nc.tensor.matmul(
                    ps[:, js],
                    w_sb[:, d, 128 * hp : 128 * hp + 128],
                    xts[d][:, js],
                    start=(d == 0),
                    stop=(d == 7),
                )
        cols = slice(1024 * half, 1024 * half + 1024)
        if drain == "scalar":
            nc.scalar.copy(dst[hp][:, cols], ps)
        else:
            nc.vector.tensor_copy(out=dst[hp][:, cols], in_=ps)

    def _vproj(xts, si0, ptag):
        """Project v for key tiles si0..si0+3 (one psum tile, 4 col groups)."""
        ps = psum.tile([128, 1024], F32, name=f"vps_{si0}", tag=ptag)
        for idx in range(4):
            ss = si0 + idx
            for d in range(8):
                nc.tensor.matmul(
                    ps[:, 256 * idx : 256 * idx + 256],
                    xts[d][:, 128 * (ss % 8) : 128 * (ss % 8) + 128],
                    wv_sb[:, d, :],
                    start=(d == 0),
                    stop=(d == 7),
                    skip_group_check=True,
                )
        nc.vector.tensor_copy(
            out=v_sb[:, si0 : si0 + 4, :, 0:DK],
            in_=ps.rearrange("p (s h d) -> p s h d", s=4, h=4),
        )

    # ---------------- attention machinery ----------------
    SC_TAGS = ["P0", "P1"]
    OT_TAGS = ["P2", "P3"]
    state = {"sc": 0, "ot": 0}

    def _scores(hp, ch, si, mask_engine="vector"):
        """Score tiles + exp + diag mask for both heads of pair hp.  The
        mask-mul is Scalar-paced (it reads the exp output), so on whichever
        engine it runs it parks in that engine's in-order queue until the exp
        lands; masks are routed (vector vs gpsimd) so they never queue ahead
        of the next pair's psum-release ops on DVE."""
        ch_lo = 1024 * ch
        t_min = 128 * si
        off = max(t_min - ch_lo, 0)
        banks = [tj for tj in (2 * ch, 2 * ch + 1) if 512 * tj + 512 > t_min]
        eng = nc.vector if mask_engine == "vector" else nc.gpsimd
        res = {}
        for h in (2 * hp, 2 * hp + 1):
            hr = 64 * (h % 2)
            scs = psum.tile(
                [128, 1024], F32, name=f"sc_{h}_{si}_{ch}", tag=SC_TAGS[state["sc"] % 2]
            )
            state["sc"] += 1
            for tj in banks:
                a = max(512 * tj, t_min)
                rel = slice(a - ch_lo, 512 * tj - ch_lo + 512)
                nc.tensor.matmul(
                    scs[:, rel],
                    kT[hp][hr : hr + 64, t_min : t_min + 128],
                    qT[hp][hr : hr + 64, a : 512 * tj + 512],
                    start=True,
                    stop=True,
                )
            att = attn_pool.tile([128, 1024], DT, name=f"at_{h}_{si}_{ch}", tag="at")
            nc.scalar.activation(
                att[:, off:1024], scs[:, off:1024], AF.Exp, scale=0.125
            )
            if mask_engine is not None and ch_lo <= t_min < ch_lo + 1024:
                eng.tensor_mul(
                    att[:, off : off + 128], att[:, off : off + 128], mtri_sb
                )
            res[h] = att
        return res

    def _mask(avatts, si, ch):
        """Deferred diag mask (DVE): emitted only after the preceding pair's
        psum-release ops so it never head-of-line-blocks them in the DVE
        queue (it is Scalar-paced: it waits on its exp)."""
        off = max(128 * si - 1024 * ch, 0)
        for h, att in avatts.items():
            nc.vector.tensor_mul(
                att[:, off : off + 128], att[:, off : off + 128], mtri_sb
            )

    def _emit_avs(hp, ch, ot, avsi, avatts):
        """attn@V for key tile avsi into the pair's ot accumulators."""
        ch_lo = 1024 * ch
        av_tmin = 128 * avsi
        for h in (2 * hp, 2 * hp + 1):
            for tj in (2 * ch, 2 * ch + 1):
                if 512 * tj + 512 <= av_tmin:
                    continue
                a = max(512 * tj, av_tmin)
                b = 512 * tj + 512
                nc.tensor.matmul(
                    ot[h][0:65, a - ch_lo : b - ch_lo],
                    v_sb[:, avsi, h, :],
                    avatts[h][:, a - ch_lo : b - ch_lo],
                    start=(avsi == 0),
                    stop=(avsi == 4 * tj + 3),
                    skip_group_check=True,
                )

    def _alloc_ot(hp, ch):
        return {
            h: psum.tile([128, 1024], F32, name=f"ot_{h}_{ch}", tag=OT_TAGS[h % 2])
            for h in (2 * hp, 2 * hp + 1)
        }

    def _normalize(hp, ch, ot, rel, mul_engine="gpsimd"):
        """outTn[.., rel cols of chunk ch] = ot rows / denominator row, for
        both heads of the pair.  The psum is released by just two DVE ops
        (raw copy + a 32-row transpose straight out of psum; rows 65:96 are
        junk and only the denominator row's transposed column is used).  The
        reciprocal runs on 32 DVE lanes; the broadcast and in-place scale of
        outTn run on gpsimd, off every critical path (gpsimd has multi-us
        op-turnaround latency, so it must never gate psum slots or the PE)."""
        ch_lo = 1024 * ch
        n = rel.stop - rel.start
        cols = slice(ch_lo + rel.start, ch_lo + rel.stop)
        for h in (2 * hp, 2 * hp + 1):
            hr = 64 * (h % 2)
            nc.vector.tensor_copy(out=outTn[hp][hr : hr + 64, cols], in_=ot[h][0:64, rel])
            dtt = bc_pool.tile([32, 1024], F32, name=f"dtt_{h}_{ch}", tag="dtt")
            nc.vector.transpose(dtt[:, 0:n], ot[h][64:96, rel])
            c0 = dtt.rearrange("p (b c) -> p b c", c=32)[:, 0 : n // 32, 0:1]
            nc.vector.reciprocal(c0, c0)
            dtr = bc_pool.tile([32, 1024], F32, name=f"dtr_{h}_{ch}", tag="dtr")
            nc.vector.transpose(dtr[:, 0:n], dtt[:, 0:n])
            # full-128-partition broadcast so the scale's two SB inputs share
            # a base partition (verifier NCC_IBIR297)
            bcb = bc_pool.tile([128, 1024], F32, name=f"bcb_{h}_{ch}", tag="bc")
            nc.gpsimd.partition_broadcast(bcb[:, 0:n], dtr[0:1, 0:n])
            nc.vector.tensor_mul(
                outTn[hp][hr : hr + 64, cols],
                outTn[hp][hr : hr + 64, cols],
                bcb[hr : hr + 64, 0:n],
            )

    def _outproj(tt, tags, drain):
        """out[t-tile, :] = outTn[:, t-tile].T @ Wo (both 128-row halves)."""
        ts = slice(128 * tt, 128 * tt + 128)
        key = "ot" if tags is OT_TAGS else "sc"
        ps = psum.tile([128, D], F32, name=f"op_{tt}", tag=tags[state[key] % 2])
        state[key] += 1
        for kk in range(2):
            for nj in range(2):
                js = slice(512 * nj, 512 * nj + 512)
                nc.tensor.matmul(
                    ps[:, js],
                    outTn[kk][:, ts],
                    wo_sb[:, kk, js],
                    start=(kk == 0),
                    stop=(kk == 1),
                )
        osb = out_pool.tile([128, D], DT, name=f"osb_{tt}", tag="osb")
        if drain == "scalar":
            nc.scalar.copy(osb, ps)
        else:
            nc.vector.tensor_copy(out=osb, in_=ps)
        nc.sync.dma_start(out=out[ts, :], in_=osb)

    # --- half-0 q/k projections (ScalarE drains them: it is idle pre-exp) ---
    _proj(wq_sb, q0, 0, "P0", "scalar", qT, 0)
    _proj(wq_sb, q0, 1, "P1", "scalar", qT, 0)
    _proj(wk_sb, k0, 0, "P2", "scalar", kT, 0)
    _proj(wk_sb, k0, 1, "P3", "scalar", kT, 0)

    # ---------------- chunk 0 (t in [0, 1024)) ----------------
    # All 32 score tiles stream through P0/P1 at exp pace; P2/P3 meanwhile
    # run vproj, the attn@V accumulators, and the pair-1 projections (pair-0's
    # slot right after its accumulator frees so chunk 1 can start stall-free).
    atts0 = {}
    for hp in range(2):
        for si in range(8):
            atts0[(hp, si)] = _scores(hp, 0, si, mask_engine=None)

    for si in range(8):
        _mask(atts0[(0, si)], si, 0)
    _vproj(v0, 0, "P2")
    _vproj(v0, 4, "P3")

    ot = _alloc_ot(0, 0)
    for si in range(8):
        _emit_avs(0, 0, ot, si, atts0[(0, si)])
    _normalize(0, 0, ot, slice(0, 512))
    _normalize(0, 0, ot, slice(512, 1024))
    # qproj half-1 pair-0 gates the first chunk-1 exp, so it must not wait on
    # the slow ot-ch0-h0 psum release: ride the P0/P1 score rotation instead
    # (free as soon as the last chunk-0 exps drain, and chunk-0 scoring is
    # done so nothing is displaced)
    _proj(wq_sb, q1, 0, SC_TAGS[state["sc"] % 2], "vector", qT, 1)
    state["sc"] += 1
    # first chunk-1 score tiles immediately after their only producer
    # (qproj half-1 pair-0): queued ahead of kproj/avs-hp1/normalize on the
    # PE so the exp stream crosses the chunk boundary ~10us earlier (they
    # use keys < 1024, so kT half-0 suffices)
    atts = {}
    for si in range(3):
        atts[si] = _scores(0, 1, si, mask_engine=None)
    _proj(wk_sb, k1, 0, "P3", "vector", kT, 1)
    for si in range(8):
        _mask(atts0[(1, si)], si, 0)
    ot = _alloc_ot(1, 0)
    for si in range(8):
        _emit_avs(1, 0, ot, si, atts0[(1, si)])
    _normalize(1, 0, ot, slice(0, 512))
    _normalize(1, 0, ot, slice(512, 1024))

    # ---------------- chunk 1 (t in [1024, 2048)) ----------------
    # pair 0: scores interleaved with pair-1 projections and the v half-1
    # projection; attn@V trails (its accumulators wait on those P2/P3 gens).
    # All P2/P3 utility tiles must be allocated BEFORE this chunk's
    # accumulators (a later same-tag allocation waits on the accumulator's
    # release and deadlocks against the attn@V stream).
    ot = None
    avn = 0

    def _avs_upto(hp, upto):
        nonlocal avn
        while avn < upto:
            _emit_avs(hp, 1, ot, avn, atts.pop(avn))
            avn += 1

    # The pair-1 projections are pushed late enough in the PE stream that the
    # P2/P3 gens they wait on (chunk-0 accumulator releases) have already
    # freed by the time the in-order PE queue reaches them — an early emission
    # would head-of-line-block every later score matmul on that wait.
    for si in range(3, 16):
        atts[si] = _scores(0, 1, si, mask_engine=None)
        if si >= 8:
            _mask(atts[si], si, 1)
        if si == 6:
            _proj(wq_sb, q1, 1, "P2", "vector", qT, 1)
        elif si == 8:
            _proj(wk_sb, k1, 1, "P3", "vector", kT, 1)
        elif si == 9:
            _vproj(v1, 8, "P2")
        elif si == 10:
            _vproj(v1, 12, "P3")
            ot = _alloc_ot(0, 1)
        elif si == 11:
            _avs_upto(0, 4)
        elif si == 12:
            _avs_upto(0, 8)
        elif si >= 13:
            _avs_upto(0, si - 1)
        if si == 13:
            _avs_upto(0, 12)
            _normalize(0, 1, ot, slice(0, 512))
    _avs_upto(0, 16)
    _normalize(0, 1, ot, slice(512, 1024))

    # pair 1: scores interleaved with the chunk-0 output projection (riding
    # the P2/P3 slots freed by pair-0's normalize) and a lagged attn@V
    # stream (its accumulators only free up after the chunk-0 outproj).
    atts = {}
    ot = None
    avn = 0
    for si in range(16):
        atts[si] = _scores(1, 1, si, mask_engine=None)
        if si >= 8:
            _mask(atts[si], si, 1)
        if 1 <= si <= 8:
            _outproj(si - 1, OT_TAGS, "vector")
        if si == 8:
            ot = _alloc_ot(1, 1)
            _avs_upto(1, 2)
        elif si >= 9:
            _avs_upto(1, 2 * (si - 7))
        if si == 13:
            _normalize(1, 1, ot, slice(0, 512))
    _avs_upto(1, 16)
    # final normalize's scale on DVE: gpsimd's op latency would land on the
    # tail, and DVE is idle by now
    _normalize(1, 1, ot, slice(512, 1024), mul_engine="vector")

    # chunk-1 output projection: P0/P1 rotation behind the last score tiles,
    # drained on ScalarE (idle once the exps are done).
    for tt in range(8, 16):
        _outproj(tt, SC_TAGS, "scalar" if tt % 2 == 0 else "vector")

    for pool in (psum, out_pool, bc_pool, attn_pool, xt_pool, persist, consts):
        pool.release()


def _get_nc():
    global _CACHED_NC
    if _CACHED_NC is None:
        nc = bacc.Bacc("TRN2", target_bir_lowering=False, debug=False)
        with tile.TileContext(nc) as tc:
            _build_kernel(tc)
        nc.compile()
        _CACHED_NC = nc
    return _CACHED_NC


def kernel(Q, K, V, mask, Wq, Wk, Wv, Wo, bo):
    global LAST_EXEC_NS
    nc = _get_nc()
    mtri = np.triu(np.ones((128, 128), dtype=np.float32))
    in_maps = []
    for c in range(8):
        b, hg = c // 4, c % 4
        rs = slice(HL * hg, HL * hg + HL)
        in_maps.append(
            {
                "qt": _prep(np.asarray(Q, np.float32)[b].T),
                "kt": _prep(np.asarray(K, np.float32)[b].T),
                "vt": _prep(np.asarray(V, np.float32)[b].T),
                "wqt": _prep_w(np.asarray(Wq, np.float32)[rs].T),
                "wkt": _prep_w(np.asarray(Wk, np.float32)[rs].T),
                "wvt": _prep_w(np.asarray(Wv, np.float32)[rs].T),
                "wot": _prep_w(np.asarray(Wo, np.float32)[:, rs].T),
                "mtri": _prep(mtri),
            }
        )
    trace = os.environ.get("BASS_MHA_TRACE", "") == "1"
    res = run_bass_kernel_spmd(nc, in_maps, core_ids=list(range(8)), trace=trace)
    LAST_EXEC_NS = res.exec_time_ns
    outs = [np.asarray(res.results[c]["out"], np.float32) for c in range(8)]
    bo = np.asarray(bo, np.float32)
    full = np.stack(
        [
            outs[0] + outs[1] + outs[2] + outs[3] + bo,
            outs[4] + outs[5] + outs[6] + outs[7] + bo,
        ]
    ).astype(np.float32)
    return full
